# revision 1
# baseline (speedup 1.0000x reference)
"""Bass/Trainium2 kernel for nn_BoxFilter: 9x9 circular box-mean over
(8, 3, 1024, 1024) f32, data-parallel across 8 NeuronCores (1 image/core).

Pipeline per core, per channel, in blocks of 120 output rows:
  - input arrives as bf16 hi/lo pairs (packed host-side during sharding;
    same 4 B/pixel DMA volume as fp32, fp32-accurate after PSUM accumulate)
  - vertical pass: banded ones-matmuls on PE (hi + lo accumulate in PSUM)
  - 1/81 scaling folded into the ACT PSUM->SBUF copy
  - horizontal pass: one DVE tensor_tensor_scan running-box recurrence
    state[t] = state[t-1] + u[t] - u[t-9] over a wrap-padded row buffer
  - loads issue on the Sync HWDGE ring, stores on the Scalar ring, with
    blocks paired into ~1 MB transfers.
"""

import numpy as np
import ml_dtypes

import concourse.bacc as bacc
import concourse.mybir as mybir
import concourse.tile as tile
from concourse.ap import AP
from concourse.bass_utils import run_bass_kernel_spmd

B, C, H, W = 8, 3, 1024, 1024
R = 4            # filter radius
WIN = 2 * R + 1  # 9
AREA = WIN * WIN
MBLK = 120       # output rows per block (input rows = MBLK + 2R = 128)
NBLK = (H + MBLK - 1) // MBLK  # 9 (last block has 64 rows)
UW = WIN + W + 2 * R  # u buffer: [9 zeros | left wrap 4 | row 1024 | right wrap 4]

_CACHE: dict = {}


def _band_weights() -> np.ndarray:
    w = np.zeros((128, MBLK), dtype=ml_dtypes.bfloat16)
    for m in range(MBLK):
        w[m : m + WIN, m] = 1.0
    return w


def _pack_image(x: np.ndarray) -> np.ndarray:
    """[C,H,W] f32 -> [C,H,2,W] bf16 (hi, lo) with hi+lo ~= x."""
    hi = x.astype(ml_dtypes.bfloat16)
    lo = (x - hi.astype(np.float32)).astype(ml_dtypes.bfloat16)
    return np.ascontiguousarray(np.stack([hi, lo], axis=2))


def _build():
    f32 = mybir.dt.float32
    bf16 = mybir.dt.bfloat16
    nc = bacc.Bacc("TRN2", target_bir_lowering=False, debug=False, num_devices=B)
    x_d = nc.dram_tensor("x", [C, H, 2, W], bf16, kind="ExternalInput")
    w_d = nc.dram_tensor("w", [128, MBLK], bf16, kind="ExternalInput")
    o_d = nc.dram_tensor("o", [C, H, W], f32, kind="ExternalOutput")
    XROW = 2 * W              # one packed image row (bf16 elements)
    XCH = H * XROW

    def vertical(v_t, x_t, w_t, m, k, q):
        for n in range(0, W, 512):
            for s in range(2):
                nc.tensor.matmul(
                    v_t[0:m, n : n + 512],
                    w_t[0:k, 0:m],
                    x_t[0:k, q, s * W + n : s * W + n + 512],
                    start=(s == 0),
                    stop=(s == 1),
                )

    def horizontal(o_t, v_t, u_t, m, oq):
        """u = [zeros(9) | v[1020:]/81 | v/81 | v[:4]/81]; one DVE box scan."""
        nc.vector.memset(u_t[0:m, 0:WIN], 0.0)
        nc.scalar.mul(out=u_t[0:m, WIN : WIN + R], in_=v_t[0:m, W - R : W], mul=1.0 / AREA)
        nc.scalar.mul(out=u_t[0:m, WIN + R + W : UW], in_=v_t[0:m, 0:R], mul=1.0 / AREA)
        nc.scalar.mul(out=u_t[0:m, WIN + R : WIN + R + W], in_=v_t[0:m, :], mul=1.0 / AREA)
        nc.vector.tensor_tensor_scan(
            out=o_t[0:m, oq, :],
            data0=u_t[0:m, WIN:UW],
            data1=u_t[0:m, 0 : UW - WIN],
            initial=0.0,
            op0=mybir.AluOpType.add,
            op1=mybir.AluOpType.subtract,
        )

    with tile.TileContext(nc) as tc:
        with (
            tc.tile_pool(name="wpool", bufs=1) as wpool,
            tc.tile_pool(name="xpool", bufs=8) as xpool,
            tc.tile_pool(name="x8pool", bufs=2) as x8pool,
            tc.tile_pool(name="o8pool", bufs=2) as o8pool,
            tc.tile_pool(name="upool", bufs=10) as upool,
            tc.tile_pool(name="opool", bufs=7) as opool,
            tc.tile_pool(name="psum", bufs=4, space="PSUM") as psum,
        ):
            w_t = wpool.tile([128, MBLK], bf16)
            nc.sync.dma_start(w_t[:], w_d.ap())

            def do_block8(c):
                m, k = H - 8 * MBLK, H - 8 * MBLK + 2 * R
                r0 = 8 * MBLK - R
                x8_t = x8pool.tile([128, 1, 2 * W], bf16, tag="x1")
                eng8 = nc.scalar if c == 0 else nc.sync
                eng8.dma_start(x8_t[0 : H - r0, 0, :], x_d.ap()[c, r0:H, :, :])
                eng8.dma_start(
                    x8_t[H - r0 : k, 0, :], x_d.ap()[c, 0 : k - (H - r0), :, :]
                )
                o8_t = o8pool.tile([MBLK, 1, W + 2 * R], f32, tag="o1")
                v_t = psum.tile([MBLK, W], f32, tag="v")
                vertical(v_t, x8_t, w_t, m, k, 0)
                u_t = upool.tile([128, UW], f32, tag="u")
                horizontal(o8_t, v_t, u_t, m, 0)
                nc.gpsimd.dma_start(
                    o_d.ap()[c, 8 * MBLK : H, :], o8_t[0:m, 0, 2 * R : 2 * R + W]
                )

            def do_pair(c, j):
                r0 = 240 * j - R
                x_t = xpool.tile([128, 2, 2 * W], bf16, tag="x2")
                if j == 0:
                    nc.sync.dma_start(x_t[0:R, 0, :], x_d.ap()[c, H - R : H, :, :])
                    nc.sync.dma_start(x_t[R:64, 0, :], x_d.ap()[c, 0 : 64 - R, :, :])
                    nc.scalar.dma_start(
                        x_t[64:128, 0, :], x_d.ap()[c, 64 - R : 128 - R, :, :]
                    )
                    nc.sync.dma_start(
                        x_t[0:64, 1, :], x_d.ap()[c, MBLK - R : MBLK - R + 64, :, :]
                    )
                    nc.scalar.dma_start(
                        x_t[64:128, 1, :],
                        x_d.ap()[c, MBLK - R + 64 : MBLK - R + 128, :, :],
                    )
                else:
                    nc.sync.dma_start(
                        x_t[:],
                        AP(
                            x_d,
                            c * XCH + r0 * XROW,
                            [[XROW, 128], [MBLK * XROW, 2], [1, XROW]],
                        ),
                    )
                o_t = opool.tile([MBLK, 2, W + 2 * R], f32, tag="o2")
                for q in range(2):
                    v_t = psum.tile([MBLK, W], f32, tag="v")
                    vertical(v_t, x_t, w_t, MBLK, 128, q)
                    u_t = upool.tile([128, UW], f32, tag="u")
                    horizontal(o_t, v_t, u_t, MBLK, q)
                nc.scalar.dma_start(
                    o_d.ap()[c, 2 * j * MBLK : (2 * j + 1) * MBLK, :],
                    o_t[:, 0, 2 * R : 2 * R + W],
                )
                nc.gpsimd.dma_start(
                    o_d.ap()[c, (2 * j + 1) * MBLK : (2 * j + 2) * MBLK, :],
                    o_t[:, 1, 2 * R : 2 * R + W],
                )

            # round-robin channels per step: uniform load/store streaming
            for c in range(C):
                do_block8(c)
            for j in range(4):
                for c in range(C):
                    do_pair(c, j)
    nc.compile()
    return nc


def _get_nc():
    if "nc" not in _CACHE:
        _CACHE["nc"] = _build()
    return _CACHE["nc"]


def _prepare_in_maps(tensor: np.ndarray) -> list:
    x = np.asarray(tensor, dtype=np.float32)
    assert x.shape == (B, C, H, W), x.shape
    wmat = _band_weights()
    return [{"x": _pack_image(x[i]), "w": wmat} for i in range(B)]


def kernel(tensor: np.ndarray) -> np.ndarray:
    nc = _get_nc()
    in_maps = _prepare_in_maps(tensor)
    res = run_bass_kernel_spmd(nc, in_maps, core_ids=list(range(B)))
    return np.stack([res.results[i]["o"] for i in range(B)], axis=0)



# revision 3
# speedup vs baseline: 1.0096x; 1.0096x over previous
"""Bass/Trainium2 kernel for nn_BoxFilter: 9x9 circular box-mean over
(8, 3, 1024, 1024) f32, data-parallel across 8 NeuronCores (1 image/core).

Strategy (measured on HW, iterated via neuron-profile):
  - rel-err budget is 2e-2, so everything moves as bf16: input is
    pre-scaled by 1/81 and downcast on host (2 B/px), output leaves
    the device as bf16 and is upcast on host: ~12.8MB HBM/core.
  - horizontal 9-box via a CUSTOM DVE op (bubble-free prefix fold
    scan(ADD, Src0 - Src1), ~1.2us per 128x1032-block vs 2.3us for the
    stock tensor_tensor_scan). Rows stream through zeroed tile margins
    (memset once on persistent tiles), so there are no per-block pad
    copies; the 8 circular-wrap output columns are completed by two
    in-place DVE adds per group using the scan's own prefix/suffix
    partial sums.
  - vertical 9-tap sum as a banded-ones bf16 matmul (PE, full 128-col
    weight so FWL engages), accumulated in PSUM f32; one PSUM->bf16
    body copy per 120-row block on ACT.
  - ~1MB loads on the Sync HWDGE ring, per-block stores on the GpSimd
    ring; 120-row blocks in 4-block groups + a 64-row tail per channel.
"""

import numpy as np
import ml_dtypes

import concourse.bacc as bacc
import concourse.mybir as mybir
import concourse.tile as tile
from concourse.ap import AP
from concourse.bass_utils import run_bass_kernel_spmd
from concourse.dve_ops import DveOp, OPS, CUSTOM_DVE_SPECS, _SUB_OPCODE_FOR_NAME
from concourse.dve_spec import Spec, Src0, Src1, AluOp, scan, lower, _has_src1
from concourse.dve_uop import DveOpSpec


def _register_box9():
    """Custom DVE op: out[k] = sum_{j<=k} (in0[j] - in1[j]) — the running
    9-box recurrence as a bubble-free prefix fold (~1.4 cyc/elem vs the
    stock tensor_tensor_scan's ~2.7)."""
    name = "BOX9_SCAN_DIFF"
    if name in _SUB_OPCODE_FOR_NAME:
        return next(op for op in OPS if op.name == name)
    spec = Spec(
        body=scan(AluOp.ADD, Src0 - Src1),
        reference=lambda in0, in1, s0, s1, imm2: np.add.accumulate(
            in0.astype(np.float32) - in1.astype(np.float32), axis=-1
        ),
    )
    row = max(_SUB_OPCODE_FOR_NAME.values()) + 1
    shas = {}
    for ver in ("v3", "v4"):
        uops = lower(spec, ver=ver)
        shas[ver] = DveOpSpec(
            name=name, opcode=row, uops=uops, rd1_en=_has_src1(spec)
        ).sha(ver)
    op = DveOp(name, spec, subdim=False, uops_sha=shas)
    OPS.append(op)
    CUSTOM_DVE_SPECS[name] = spec
    _SUB_OPCODE_FOR_NAME[name] = row
    return op


BOX9 = _register_box9()

B, C, H, W = 8, 3, 1024, 1024
R = 4            # filter radius
WIN = 2 * R + 1  # 9
AREA = WIN * WIN
MBLK = 120                  # output rows per block (input rows = 128)
TAILM = H - 8 * MBLK        # 64 output rows in the tail block
XL = 1056                   # x tile line length per chunk
SCW = W + 2 * R             # scan width: 1032 outputs
NBLK = 9                    # blocks per channel (8 full + tail)

_CACHE: dict = {}


def _band_weights() -> np.ndarray:
    w = np.zeros((128, 128), dtype=ml_dtypes.bfloat16)
    for m in range(MBLK):
        w[m : m + WIN, m] = 1.0
    return w


def _build():
    f32 = mybir.dt.float32
    bf16 = mybir.dt.bfloat16
    nc = bacc.Bacc("TRN2", target_bir_lowering=False, debug=False, num_devices=B)
    x_d = nc.dram_tensor("x", [C, H, W], bf16, kind="ExternalInput")
    w_d = nc.dram_tensor("w", [128, 128], bf16, kind="ExternalInput")
    o_d = nc.dram_tensor("o", [C, H, W], bf16, kind="ExternalOutput")

    with tile.TileContext(nc) as tc:
        with (
            tc.tile_pool(name="wpool", bufs=1) as wpool,
            tc.tile_pool(name="xpool", bufs=4) as xpool,
            tc.tile_pool(name="xtpool", bufs=2) as xtpool,
            tc.tile_pool(name="hpool", bufs=4) as hpool,
            tc.tile_pool(name="opool", bufs=8) as opool,
            tc.tile_pool(name="otpool", bufs=2) as otpool,
            tc.tile_pool(name="psum", bufs=4, space="PSUM") as psum,
        ):
            w_t = wpool.tile([128, 128], bf16)

            # persistent x tiles: margins zeroed once, reused round-robin
            xts = [xpool.tile([128, 4, XL], bf16, name=f"xg{i}") for i in range(4)]
            xtt = [xtpool.tile([TAILM + 2 * R, 1, XL], bf16, name=f"xt{i}") for i in range(2)]
            for t in xts:
                tf = t[:, 0:4, 0:XL].rearrange("p a b -> p (a b)")
                nc.gpsimd.memset(tf[:, 0:WIN], 0.0)
                for q in range(3):
                    nc.gpsimd.memset(
                        tf[:, q * XL + WIN + W : (q + 1) * XL + WIN], 0.0
                    )
                nc.gpsimd.memset(tf[:, 3 * XL + WIN + W : 4 * XL], 0.0)
            for t in xtt:
                nc.gpsimd.memset(t[:, 0, 0:WIN], 0.0)
                nc.gpsimd.memset(t[:, 0, WIN + W : WIN + W + 2 * R], 0.0)

            def scan_chunk(x_t, hx, q, k):
                nc.vector._custom_dve(
                    BOX9,
                    out=hx[0:k, q, 0:SCW],
                    in0=x_t[0:k, q, WIN : WIN + SCW],
                    in1=x_t[0:k, q, 0:SCW],
                )

            def wrap_fix(hx, k, nq):
                """Complete the 8 circular-wrap centers per chunk by adding
                the prefix/suffix partial sums in place (2D across chunks)."""
                nc.vector.tensor_tensor(
                    out=hx[0:k, 0:nq, R : 2 * R],
                    in0=hx[0:k, 0:nq, R : 2 * R],
                    in1=hx[0:k, 0:nq, 1024 + R : 1024 + 2 * R],
                    op=mybir.AluOpType.add,
                )
                nc.vector.tensor_tensor(
                    out=hx[0:k, 0:nq, 1024 : 1024 + R],
                    in0=hx[0:k, 0:nq, 1024 : 1024 + R],
                    in1=hx[0:k, 0:nq, 0:R],
                    op=mybir.AluOpType.add,
                )

            def block_matmuls(hx, v_t, vq, q, k):
                wk = w_t[0:k, :]
                nc.tensor.matmul(
                    v_t[:, vq, 0:512], wk, hx[0:k, q, R : R + 512],
                    start=True, stop=True,
                )
                nc.tensor.matmul(
                    v_t[:, vq, 512:1024], wk, hx[0:k, q, R + 512 : R + 1024],
                    start=True, stop=True,
                )

            def do_group(c, g, x_t):
                """Four 120-row blocks: one ~1MB load, one ~1MB store."""
                r0 = 480 * g - R
                if g == 0:
                    nc.sync.dma_start(
                        x_t[:, 1:4, WIN : WIN + W],
                        AP(
                            x_d,
                            c * H * W + (MBLK - R) * W,
                            [[W, 128], [MBLK * W, 3], [1, W]],
                        ),
                    )
                    nc.sync.dma_start(
                        x_t[0:R, 0, WIN : WIN + W],
                        AP(x_d, c * H * W + (H - R) * W, [[W, R], [1, W]]),
                    )
                    nc.sync.dma_start(
                        x_t[R:128, 0, WIN : WIN + W],
                        AP(x_d, c * H * W, [[W, 128 - R], [1, W]]),
                    )
                    if c == 0:
                        nc.sync.dma_start(w_t[:], w_d.ap())
                else:
                    nc.sync.dma_start(
                        x_t[:, 0:4, WIN : WIN + W],
                        AP(x_d, c * H * W + r0 * W, [[W, 128], [MBLK * W, 4], [1, W]]),
                    )
                qorder = (1, 2, 3, 0) if (c == 0 and g == 0) else (0, 1, 2, 3)
                hx = hpool.tile([128, 4, XL], bf16, tag="hx")
                for q in qorder:
                    scan_chunk(x_t, hx, q, 128)
                wrap_fix(hx, 128, 4)
                for q in qorder:
                    v_t = psum.tile([128, 1, W], f32, tag="v")
                    block_matmuls(hx, v_t, 0, q, 128)
                    o_t = opool.tile([MBLK, 1, W], bf16, tag="o")
                    nc.scalar.copy(o_t[0:MBLK, 0, :], v_t[0:MBLK, 0, :])
                    nc.gpsimd.dma_start(
                        AP(
                            o_d,
                            c * H * W + (480 * g + MBLK * q) * W,
                            [[W, MBLK], [1, W]],
                        ),
                        o_t[:, 0, :],
                    )

            def do_tail(c, x_t):
                """Last 64 output rows (input rows 956..1023 + wrap 0..3)."""
                k = TAILM + 2 * R  # 72
                r0 = 8 * MBLK - R  # 956
                nc.sync.dma_start(
                    x_t[0 : H - r0, 0, WIN : WIN + W],
                    AP(x_d, c * H * W + r0 * W, [[W, H - r0], [1, W]]),
                )
                nc.sync.dma_start(
                    x_t[H - r0 : k, 0, WIN : WIN + W],
                    AP(x_d, c * H * W, [[W, k - (H - r0)], [1, W]]),
                )
                o_t = otpool.tile([TAILM, 1, W], bf16, tag="ot")
                v_t = psum.tile([128, 1, W], f32, tag="v")
                hx = hpool.tile([128, 4, XL], bf16, tag="hx")
                scan_chunk(x_t, hx, 0, k)
                wrap_fix(hx, k, 1)
                block_matmuls(hx, v_t, 0, 0, k)
                nc.scalar.copy(o_t[0:TAILM, 0, :], v_t[0:TAILM, 0, :])
                nc.gpsimd.dma_start(
                    AP(o_d, c * H * W + 8 * MBLK * W, [[W, TAILM], [1, W]]),
                    o_t[:, 0, :],
                )

            gi = 0
            for c in range(C):
                do_group(c, 0, xts[gi % 4])
                gi += 1
                do_tail(c, xtt[c % 2])
                do_group(c, 1, xts[gi % 4])
                gi += 1
    nc.compile()
    return nc


def _get_nc():
    if "nc" not in _CACHE:
        _CACHE["nc"] = _build()
    return _CACHE["nc"]


def _prepare_in_maps(tensor: np.ndarray) -> list:
    x = np.asarray(tensor, dtype=np.float32)
    assert x.shape == (B, C, H, W), x.shape
    xs = (x * np.float32(1.0 / AREA)).astype(ml_dtypes.bfloat16)
    wmat = _band_weights()
    return [{"x": xs[i], "w": wmat} for i in range(B)]


def kernel(tensor: np.ndarray) -> np.ndarray:
    nc = _get_nc()
    in_maps = _prepare_in_maps(tensor)
    res = run_bass_kernel_spmd(nc, in_maps, core_ids=list(range(B)))
    return np.stack(
        [res.results[i]["o"].astype(np.float32) for i in range(B)], axis=0
    )
